# revision 34
# baseline (speedup 1.0000x reference)
"""Trainium2 kernel for nn_DistanceLoss (retrieval_knn, bs=1, N=16384).

reference semantics (sym branch, model_index in (0,)):
    p = R @ pts_model + t                      # (N, 3) predicted points
    d2[i, j] = ||p_i - g_j||^2                 # (N, N) vs ground-truth points
    loss = mean_i sqrt(min_j d2[i, j])         # scalar, shape (1,)

Key identity: sqrt(d2[i, argmin_j]) == sqrt(min_j d2[i, j]), so no
argmin/gather is needed — only a min-reduction over the distance matrix:
    min_j d2[i, j] = p_i^2 + min_j (g_j^2 - 2 p_i . g_j)

Device work (the O(N^2) part), sharded over 8 cores on the pred axis:
  - PE: S[i, j] = -2 p_i . g_j + g_j^2 as a K=11 matmul: each fp32 operand
    is split into fp16 hi/lo halves (lo scaled by 2^6 to dodge fp16
    subnormals, compensated on the other operand) so every partial product
    is exact in the fp32 PSUM accumulate; abs error ~1e-5.
  - Reduction of the 2048x16384 S-slice per core: ScalarE copies every even
    PSUM group to SBUF; a custom fused DVE op (MIN_TT_REDUCE_ANT:
    out = min(in0, in1), accum_out = min(s0, reduce_min(out))) consumes
    (odd PSUM group, even SBUF group) pairs at 1 result/cycle = 2 source
    elements/cycle. Per-pair accumulators land in a [128, 8] tile reduced
    once per block, keeping all fused ops independent for the scheduler.
Host work (O(N)): pose transform, fp16 feature split, final p^2 add +
sqrt + mean in float64, and the trivial non-symmetric branch.
"""

import numpy as np

N_PTS = 16384
N_CORES = 8
SYM_LIST = (0,)

PRED_PER_CORE = N_PTS // N_CORES          # 2048
N_BLOCKS = PRED_PER_CORE // 128           # 16 pred blocks of 128 rows
GROUP = 1024                              # gt points per PSUM group (2 banks)
N_GROUPS = N_PTS // GROUP                 # 16 groups -> 8 (even, odd) pairs
N_PAIRS = N_GROUPS // 2
N_CHAINS = N_PAIRS                        # one accumulator column per pair
K_ROWS = 11                               # fp16 split rows (3 per coord + 2)
LO_SCALE = np.float32(64.0)               # 2^6 subnormal-dodge scale

TRACE = False          # test.py sets True to capture a profiled run
LAST_RESULT = None     # BassKernelResults of the most recent device run

_COMPILED = None


def _register_min_ttr():
    """Register a custom fused DVE op:
        out = min(in0, in1);  accum_out = min(reduce_min(out), s0)
    One DVE instruction consumes TWO tiles at 1 result/cycle — 2x the
    throughput of tensor_reduce for the same reduction work. (The native
    TENSOR_TENSOR_REDUCE opcode crashes this runtime's exec unit; the
    table-driven custom-DVE path works.)"""
    from concourse.dve_spec import Spec, Src0, Src1, C0, minn, lower, _has_src1
    from concourse.dve_uop import DveOpSpec
    from concourse import dve_ops

    name = "MIN_TT_REDUCE_ANT"
    for o in dve_ops.OPS:
        if o.name == name:
            return o

    def _ref(in0, in1, c0, c1, c2):
        b = np.minimum(in0.astype(np.float32), in1.astype(np.float32))
        acc = np.minimum(
            np.float32(c0), b.reshape(b.shape[0], -1).min(axis=-1, keepdims=True)
        )
        return b, acc

    spec = Spec(body=minn(Src0, Src1), accum=minn, accum_init=C0, reference=_ref)
    row = max(dve_ops._SUB_OPCODE_FOR_NAME.values()) + 1
    dve_ops._SUB_OPCODE_FOR_NAME[name] = row
    shas = {}
    for ver in ("v3", "v4"):
        uops = lower(spec, ver=ver)
        shas[ver] = DveOpSpec(
            name=name, opcode=row, uops=uops, rd1_en=_has_src1(spec)
        ).sha(ver)
    op = dve_ops.DveOp(name, spec, subdim=False, uops_sha=shas)
    dve_ops.OPS.append(op)
    dve_ops.CUSTOM_DVE_SPECS[name] = spec
    return op


def _build_module():
    import concourse.bacc as bacc
    import concourse.tile as tile
    import concourse.mybir as mybir

    f16 = mybir.dt.float16
    f32 = mybir.dt.float32
    min_ttr = _register_min_ttr()

    nc = bacc.Bacc(
        "TRN2", target_bir_lowering=False, debug=False, num_devices=N_CORES
    )
    lhsT = nc.dram_tensor("lhsT", [K_ROWS, PRED_PER_CORE], f16, kind="ExternalInput")
    rhs = nc.dram_tensor("rhs", [K_ROWS, N_PTS], f16, kind="ExternalInput")
    out = nc.dram_tensor("out", [128, N_BLOCKS], f32, kind="ExternalOutput")

    with tile.TileContext(nc) as tc:
        with (
            tc.tile_pool(name="consts", bufs=1) as consts,
            tc.tile_pool(name="scrp", bufs=6) as scrp,
            tc.tile_pool(name="ttrop", bufs=4) as ttrop,
            tc.tile_pool(name="accp", bufs=12) as accp,
            tc.tile_pool(name="ps", bufs=4, space="PSUM") as psp,
        ):
            # features replicated at partition offsets 0/32/64/96 so four
            # K=11 matmuls run CONCURRENTLY in distinct PE row-groups.
            # rhs split into free-quarter tiles so early matmuls only
            # wait on the first quarter of the DMA.
            bounds = [0, 4096, 8192, 12288, N_PTS]
            lhs_sb = consts.tile([96 + K_ROWS, PRED_PER_CORE], f16)
            rhs_tiles = [
                consts.tile(
                    [96 + K_ROWS, bounds[q + 1] - bounds[q]],
                    f16,
                    name=f"rhs_sb{q}",
                )
                for q in range(len(bounds) - 1)
            ]
            outbuf = consts.tile([128, N_BLOCKS], f32)
            for r in range(4):
                nc.sync.dma_start(lhs_sb[32 * r : 32 * r + K_ROWS, :], lhsT[:])
            for q in range(len(bounds) - 1):
                w = bounds[q + 1] - bounds[q]
                # quarter 0 gates the first matmuls: split each replica DMA
                # into two parallel-queue halves to halve its latency
                nsplit = 2 if q == 0 else 1
                for r in range(4):
                    p0 = 32 * r
                    for s in range(nsplit):
                        c0 = s * (w // nsplit)
                        c1 = (s + 1) * (w // nsplit)
                        nc.sync.dma_start(
                            rhs_tiles[q][p0 : p0 + K_ROWS, c0:c1],
                            rhs[:, bounds[q] + c0 : bounds[q] + c1],
                        )

            def rhs_slice(c):
                for q in range(len(bounds) - 1):
                    if c < bounds[q + 1]:
                        return rhs_tiles[q], c - bounds[q]
                raise AssertionError(c)

            # warm-up: absorb one-time ACT/DVE table-load penalties
            # while the DMAs stream (no dependency on inputs)
            warm = scrp.tile([128, 32], f32, tag="warm")
            warm2 = scrp.tile([128, 32], f32, tag="warm")
            wacc = accp.tile([128, 1], f32, tag="acc")
            nc.vector.memset(warm[:], 0.0)
            nc.scalar.copy(warm2[:], warm[:])
            nc.vector._custom_dve(
                min_ttr, out=warm2[:], in0=warm[:], in1=warm2[:],
                s0=3.0e38, accum_out=wacc[:],
            )

            n_mm = GROUP // 512

            def mm_group(ps, b, g):
                """One PSUM group: gt 512-tiles [n_mm*g, n_mm*(g+1)), spread
                over PE row-groups so matmuls run concurrently."""
                for t in range(n_mm):
                    j_tile = n_mm * g + t
                    p0 = 32 * (j_tile % 4)
                    src, co = rhs_slice(j_tile * 512)
                    nc.tensor.matmul(
                        ps[:, t * 512 : (t + 1) * 512],
                        lhs_sb[p0 : p0 + K_ROWS, b * 128 : (b + 1) * 128],
                        src[p0 : p0 + K_ROWS, co : co + 512],
                        start=True,
                        stop=True,
                        tile_position=(p0, 0),
                    )

            for b in range(N_BLOCKS):
                # every pair independent: partial mins land in a per-block
                # [128, N_CHAINS] tile, reduced once per block
                chain_accs = accp.tile([128, N_CHAINS], f32, tag="chacc")
                for k in range(N_PAIRS):
                    # even group: ScalarE copies PSUM -> SBUF
                    ps_a = psp.tile([128, GROUP], f32, tag="ps")
                    mm_group(ps_a, b, 2 * k)
                    scr = scrp.tile([128, GROUP], f32, tag="scr")
                    nc.scalar.copy(scr[:], ps_a[:])
                    # odd group: consumed straight from PSUM by the fused op
                    ps_b = psp.tile([128, GROUP], f32, tag="ps")
                    mm_group(ps_b, b, 2 * k + 1)
                    ttr_out = ttrop.tile([128, GROUP], f32, tag="ttro")
                    nc.vector._custom_dve(
                        min_ttr,
                        out=ttr_out[:],
                        in0=ps_b[:],
                        in1=scr[:],
                        s0=3.0e38,
                        accum_out=chain_accs[:, k : k + 1],
                    )
                nc.vector.tensor_reduce(
                    outbuf[:, b : b + 1],
                    chain_accs[:],
                    axis=mybir.AxisListType.X,
                    op=mybir.AluOpType.min,
                )
            nc.sync.dma_start(out[:], outbuf[:])
    nc.compile()
    return nc


def _get_module():
    global _COMPILED
    if _COMPILED is None:
        _COMPILED = _build_module()
    return _COMPILED


def _split_f16(x):
    """x (fp32) -> (hi, lo*2^6) fp16 pair with exact-product semantics."""
    hi = x.astype(np.float16)
    lo = ((x - hi.astype(np.float32)) * LO_SCALE).astype(np.float16)
    return hi, lo


def kernel(pred_R, pred_t, pts_model, pts_gt, model_index):
    global LAST_RESULT
    pred_R = np.asarray(pred_R, dtype=np.float32)
    pred_t = np.asarray(pred_t, dtype=np.float32)
    pts_model = np.asarray(pts_model, dtype=np.float32)
    pts_gt = np.asarray(pts_gt, dtype=np.float32)

    # pose transform (O(N), host): p[b,n,:] = R[b] @ model[b,n,:] + t[b]
    p = np.einsum("bij,bnj->bni", pred_R, pts_model) + pred_t[:, None, :]

    if int(model_index) not in SYM_LIST:
        diff = (p - pts_gt).astype(np.float64)
        loss = np.mean(np.sqrt(np.sum(diff * diff, axis=2)), axis=1)
        return loss.astype(np.float32)

    p = p[0]                       # (N, 3) queries
    g = pts_gt[0].astype(np.float32)   # (N, 3) references

    # features: S[i,j] = sum_k lhsT[k,i] * rhs[k,j] = -2 p.g + g^2
    a = -2.0 * p                                   # (N, 3)
    ah, al = _split_f16(a)
    gh, gl = _split_f16(g)
    c = (g.astype(np.float64) ** 2).sum(axis=1).astype(np.float32)   # g^2
    ch, cl = _split_f16(c)
    inv = np.float32(1.0) / LO_SCALE

    ones = np.ones(N_PTS, np.float16)
    # per coord: (Ah,Gh), (Al*64, Gh/64), (Ah/64, Gl*64); then (1,Ch), (1/64, Cl*64)
    lhs_rows, rhs_rows = [], []
    for ci in range(3):
        ahc = ah[:, ci]
        ghc = gh[:, ci]
        lhs_rows += [ahc, al[:, ci], (ahc.astype(np.float32) * inv).astype(np.float16)]
        rhs_rows += [ghc, (ghc.astype(np.float32) * inv).astype(np.float16), gl[:, ci]]
    lhs_rows += [ones, (ones.astype(np.float32) * inv).astype(np.float16)]
    rhs_rows += [ch, cl]
    lhs_full = np.stack(lhs_rows)                  # (11, N) fp16
    rhs_full = np.stack(rhs_rows)                  # (11, N) fp16

    nc = _get_module()
    from concourse.bass_utils import run_bass_kernel_spmd

    in_maps = []
    for core in range(N_CORES):
        sl = slice(core * PRED_PER_CORE, (core + 1) * PRED_PER_CORE)
        in_maps.append(
            {
                "lhsT": np.ascontiguousarray(lhs_full[:, sl]),
                "rhs": rhs_full,
            }
        )
    kw = {}
    if TRACE:
        kw = {"trace": True, "trace_cores": list(range(N_CORES))}
    res = run_bass_kernel_spmd(nc, in_maps, core_ids=list(range(N_CORES)), **kw)
    LAST_RESULT = res

    # assemble: out[p, b] = min_j S for pred index core*2048 + b*128 + p
    min_s = np.concatenate(
        [res.results[core]["out"].T.reshape(-1) for core in range(N_CORES)]
    ).astype(np.float64)
    p2 = (p.astype(np.float64) ** 2).sum(axis=1)
    d2 = np.maximum(p2 + min_s, 0.0)
    loss = np.mean(np.sqrt(d2))
    return np.array([loss], dtype=np.float32)


# revision 35
# speedup vs baseline: 1.0133x; 1.0133x over previous
"""Trainium2 kernel for nn_DistanceLoss (retrieval_knn, bs=1, N=16384).

reference semantics (sym branch, model_index in (0,)):
    p = R @ pts_model + t                      # (N, 3) predicted points
    d2[i, j] = ||p_i - g_j||^2                 # (N, N) vs ground-truth points
    loss = mean_i sqrt(min_j d2[i, j])         # scalar, shape (1,)

Key identity: sqrt(d2[i, argmin_j]) == sqrt(min_j d2[i, j]), so no
argmin/gather is needed — only a min-reduction over the distance matrix:
    min_j d2[i, j] = p_i^2 + min_j (g_j^2 - 2 p_i . g_j)

Device work (the O(N^2) part), sharded over 8 cores on the pred axis:
  - PE: S[i, j] = -2 p_i . g_j + g_j^2 as a K=11 matmul: each fp32 operand
    is split into fp16 hi/lo halves (lo scaled by 2^6 to dodge fp16
    subnormals, compensated on the other operand) so every partial product
    is exact in the fp32 PSUM accumulate; abs error ~1e-5.
  - Reduction of the 2048x16384 S-slice per core: ScalarE copies every even
    PSUM group to SBUF; a custom fused DVE op (MIN_TT_REDUCE_ANT:
    out = min(in0, in1), accum_out = min(s0, reduce_min(out))) consumes
    (odd PSUM group, even SBUF group) pairs at 1 result/cycle = 2 source
    elements/cycle. Per-pair accumulators land in a [128, 8] tile reduced
    once per block, keeping all fused ops independent for the scheduler.
Host work (O(N)): pose transform, fp16 feature split, final p^2 add +
sqrt + mean in float64, and the trivial non-symmetric branch.
"""

import numpy as np

N_PTS = 16384
N_CORES = 8
SYM_LIST = (0,)

PRED_PER_CORE = N_PTS // N_CORES          # 2048
N_BLOCKS = PRED_PER_CORE // 128           # 16 pred blocks of 128 rows
GROUP = 1024                              # gt points per PSUM group (2 banks)
N_GROUPS = N_PTS // GROUP                 # 16 groups -> 8 (even, odd) pairs
N_PAIRS = N_GROUPS // 2
N_CHAINS = N_PAIRS                        # one accumulator column per pair
K_ROWS = 11                               # fp16 split rows (3 per coord + 2)
LO_SCALE = np.float32(64.0)               # 2^6 subnormal-dodge scale

TRACE = False          # test.py sets True to capture a profiled run
LAST_RESULT = None     # BassKernelResults of the most recent device run

_COMPILED = None


def _register_min_ttr():
    """Register a custom fused DVE op:
        out = min(in0, in1);  accum_out = min(reduce_min(out), s0)
    One DVE instruction consumes TWO tiles at 1 result/cycle — 2x the
    throughput of tensor_reduce for the same reduction work. (The native
    TENSOR_TENSOR_REDUCE opcode crashes this runtime's exec unit; the
    table-driven custom-DVE path works.)"""
    from concourse.dve_spec import Spec, Src0, Src1, C0, minn, lower, _has_src1
    from concourse.dve_uop import DveOpSpec
    from concourse import dve_ops

    name = "MIN_TT_REDUCE_ANT"
    for o in dve_ops.OPS:
        if o.name == name:
            return o

    def _ref(in0, in1, c0, c1, c2):
        b = np.minimum(in0.astype(np.float32), in1.astype(np.float32))
        acc = np.minimum(
            np.float32(c0), b.reshape(b.shape[0], -1).min(axis=-1, keepdims=True)
        )
        return b, acc

    spec = Spec(body=minn(Src0, Src1), accum=minn, accum_init=C0, reference=_ref)
    row = max(dve_ops._SUB_OPCODE_FOR_NAME.values()) + 1
    dve_ops._SUB_OPCODE_FOR_NAME[name] = row
    shas = {}
    for ver in ("v3", "v4"):
        uops = lower(spec, ver=ver)
        shas[ver] = DveOpSpec(
            name=name, opcode=row, uops=uops, rd1_en=_has_src1(spec)
        ).sha(ver)
    op = dve_ops.DveOp(name, spec, subdim=False, uops_sha=shas)
    dve_ops.OPS.append(op)
    dve_ops.CUSTOM_DVE_SPECS[name] = spec
    return op


def _build_module():
    import concourse.bacc as bacc
    import concourse.tile as tile
    import concourse.mybir as mybir

    f16 = mybir.dt.float16
    f32 = mybir.dt.float32
    min_ttr = _register_min_ttr()

    nc = bacc.Bacc(
        "TRN2", target_bir_lowering=False, debug=False, num_devices=N_CORES
    )
    lhsT = nc.dram_tensor("lhsT", [K_ROWS, PRED_PER_CORE], f16, kind="ExternalInput")
    rhs = nc.dram_tensor("rhs", [K_ROWS, N_PTS], f16, kind="ExternalInput")
    out = nc.dram_tensor("out", [128, N_BLOCKS], f32, kind="ExternalOutput")

    with tile.TileContext(nc) as tc:
        with (
            tc.tile_pool(name="consts", bufs=1) as consts,
            tc.tile_pool(name="scrp", bufs=6) as scrp,
            tc.tile_pool(name="ttrop", bufs=4) as ttrop,
            tc.tile_pool(name="accp", bufs=12) as accp,
            tc.tile_pool(name="ps", bufs=4, space="PSUM") as psp,
        ):
            # features replicated at partition offsets 0/32/64/96 so four
            # K=11 matmuls run CONCURRENTLY in distinct PE row-groups.
            # rhs split into free-quarter tiles so early matmuls only
            # wait on the first quarter of the DMA.
            bounds = [0, 4096, 8192, 12288, N_PTS]
            lhs_sb = consts.tile([96 + K_ROWS, PRED_PER_CORE], f16)
            rhs_tiles = [
                consts.tile(
                    [96 + K_ROWS, bounds[q + 1] - bounds[q]],
                    f16,
                    name=f"rhs_sb{q}",
                )
                for q in range(len(bounds) - 1)
            ]
            outbuf = consts.tile([128, N_BLOCKS], f32)
            for r in range(4):
                nc.sync.dma_start(lhs_sb[32 * r : 32 * r + K_ROWS, :], lhsT[:])
            for q in range(len(bounds) - 1):
                for r in range(4):
                    p0 = 32 * r
                    nc.sync.dma_start(
                        rhs_tiles[q][p0 : p0 + K_ROWS, :],
                        rhs[:, bounds[q] : bounds[q + 1]],
                    )

            def rhs_slice(c):
                for q in range(len(bounds) - 1):
                    if c < bounds[q + 1]:
                        return rhs_tiles[q], c - bounds[q]
                raise AssertionError(c)

            # warm-up: absorb one-time ACT/DVE table-load penalties
            # while the DMAs stream (no dependency on inputs)
            warm = scrp.tile([128, 32], f32, tag="warm")
            warm2 = scrp.tile([128, 32], f32, tag="warm")
            wacc = accp.tile([128, 1], f32, tag="acc")
            nc.vector.memset(warm[:], 0.0)
            nc.scalar.copy(warm2[:], warm[:])
            nc.vector._custom_dve(
                min_ttr, out=warm2[:], in0=warm[:], in1=warm2[:],
                s0=3.0e38, accum_out=wacc[:],
            )

            n_mm = GROUP // 512

            def mm_group(ps, b, g):
                """One PSUM group: gt 512-tiles [n_mm*g, n_mm*(g+1)), spread
                over PE row-groups so matmuls run concurrently."""
                for t in range(n_mm):
                    j_tile = n_mm * g + t
                    p0 = 32 * (j_tile % 4)
                    src, co = rhs_slice(j_tile * 512)
                    nc.tensor.matmul(
                        ps[:, t * 512 : (t + 1) * 512],
                        lhs_sb[p0 : p0 + K_ROWS, b * 128 : (b + 1) * 128],
                        src[p0 : p0 + K_ROWS, co : co + 512],
                        start=True,
                        stop=True,
                        tile_position=(p0, 0),
                    )

            for b in range(N_BLOCKS):
                # every pair independent: partial mins land in a per-block
                # [128, N_CHAINS] tile, reduced once per block
                chain_accs = accp.tile([128, N_CHAINS], f32, tag="chacc")
                for k in range(N_PAIRS):
                    # even group: ScalarE copies PSUM -> SBUF
                    ps_a = psp.tile([128, GROUP], f32, tag="ps")
                    mm_group(ps_a, b, 2 * k)
                    scr = scrp.tile([128, GROUP], f32, tag="scr")
                    nc.scalar.copy(scr[:], ps_a[:])
                    # odd group: consumed straight from PSUM by the fused op
                    ps_b = psp.tile([128, GROUP], f32, tag="ps")
                    mm_group(ps_b, b, 2 * k + 1)
                    ttr_out = ttrop.tile([128, GROUP], f32, tag="ttro")
                    nc.vector._custom_dve(
                        min_ttr,
                        out=ttr_out[:],
                        in0=ps_b[:],
                        in1=scr[:],
                        s0=3.0e38,
                        accum_out=chain_accs[:, k : k + 1],
                    )
                nc.vector.tensor_reduce(
                    outbuf[:, b : b + 1],
                    chain_accs[:],
                    axis=mybir.AxisListType.X,
                    op=mybir.AluOpType.min,
                )
            nc.sync.dma_start(out[:], outbuf[:])
    nc.compile()
    return nc


def _get_module():
    global _COMPILED
    if _COMPILED is None:
        _COMPILED = _build_module()
    return _COMPILED


def _split_f16(x):
    """x (fp32) -> (hi, lo*2^6) fp16 pair with exact-product semantics."""
    hi = x.astype(np.float16)
    lo = ((x - hi.astype(np.float32)) * LO_SCALE).astype(np.float16)
    return hi, lo


def kernel(pred_R, pred_t, pts_model, pts_gt, model_index):
    global LAST_RESULT
    pred_R = np.asarray(pred_R, dtype=np.float32)
    pred_t = np.asarray(pred_t, dtype=np.float32)
    pts_model = np.asarray(pts_model, dtype=np.float32)
    pts_gt = np.asarray(pts_gt, dtype=np.float32)

    # pose transform (O(N), host): p[b,n,:] = R[b] @ model[b,n,:] + t[b]
    p = np.einsum("bij,bnj->bni", pred_R, pts_model) + pred_t[:, None, :]

    if int(model_index) not in SYM_LIST:
        diff = (p - pts_gt).astype(np.float64)
        loss = np.mean(np.sqrt(np.sum(diff * diff, axis=2)), axis=1)
        return loss.astype(np.float32)

    p = p[0]                       # (N, 3) queries
    g = pts_gt[0].astype(np.float32)   # (N, 3) references

    # features: S[i,j] = sum_k lhsT[k,i] * rhs[k,j] = -2 p.g + g^2
    a = -2.0 * p                                   # (N, 3)
    ah, al = _split_f16(a)
    gh, gl = _split_f16(g)
    c = (g.astype(np.float64) ** 2).sum(axis=1).astype(np.float32)   # g^2
    ch, cl = _split_f16(c)
    inv = np.float32(1.0) / LO_SCALE

    ones = np.ones(N_PTS, np.float16)
    # per coord: (Ah,Gh), (Al*64, Gh/64), (Ah/64, Gl*64); then (1,Ch), (1/64, Cl*64)
    lhs_rows, rhs_rows = [], []
    for ci in range(3):
        ahc = ah[:, ci]
        ghc = gh[:, ci]
        lhs_rows += [ahc, al[:, ci], (ahc.astype(np.float32) * inv).astype(np.float16)]
        rhs_rows += [ghc, (ghc.astype(np.float32) * inv).astype(np.float16), gl[:, ci]]
    lhs_rows += [ones, (ones.astype(np.float32) * inv).astype(np.float16)]
    rhs_rows += [ch, cl]
    lhs_full = np.stack(lhs_rows)                  # (11, N) fp16
    rhs_full = np.stack(rhs_rows)                  # (11, N) fp16

    nc = _get_module()
    from concourse.bass_utils import run_bass_kernel_spmd

    in_maps = []
    for core in range(N_CORES):
        sl = slice(core * PRED_PER_CORE, (core + 1) * PRED_PER_CORE)
        in_maps.append(
            {
                "lhsT": np.ascontiguousarray(lhs_full[:, sl]),
                "rhs": rhs_full,
            }
        )
    kw = {}
    if TRACE:
        kw = {"trace": True, "trace_cores": list(range(N_CORES))}
    res = run_bass_kernel_spmd(nc, in_maps, core_ids=list(range(N_CORES)), **kw)
    LAST_RESULT = res

    # assemble: out[p, b] = min_j S for pred index core*2048 + b*128 + p
    min_s = np.concatenate(
        [res.results[core]["out"].T.reshape(-1) for core in range(N_CORES)]
    ).astype(np.float64)
    p2 = (p.astype(np.float64) ** 2).sum(axis=1)
    d2 = np.maximum(p2 + min_s, 0.0)
    loss = np.mean(np.sqrt(d2))
    return np.array([loss], dtype=np.float32)


# revision 37
# speedup vs baseline: 1.0295x; 1.0160x over previous
"""Trainium2 kernel for nn_DistanceLoss (retrieval_knn, bs=1, N=16384).

reference semantics (sym branch, model_index in (0,)):
    p = R @ pts_model + t                      # (N, 3) predicted points
    d2[i, j] = ||p_i - g_j||^2                 # (N, N) vs ground-truth points
    loss = mean_i sqrt(min_j d2[i, j])         # scalar, shape (1,)

Key identity: sqrt(d2[i, argmin_j]) == sqrt(min_j d2[i, j]), so no
argmin/gather is needed — only a min-reduction over the distance matrix:
    min_j d2[i, j] = p_i^2 + min_j (g_j^2 - 2 p_i . g_j)

Device work (the O(N^2) part), sharded over 8 cores on the pred axis:
  - PE: S[i, j] = -2 p_i . g_j + g_j^2 as a K=11 matmul: each fp32 operand
    is split into fp16 hi/lo halves (lo scaled by 2^6 to dodge fp16
    subnormals, compensated on the other operand) so every partial product
    is exact in the fp32 PSUM accumulate; abs error ~1e-5.
  - Reduction of the 2048x16384 S-slice per core: ScalarE copies every even
    PSUM group to SBUF; a custom fused DVE op (MIN_TT_REDUCE_ANT:
    out = min(in0, in1), accum_out = min(s0, reduce_min(out))) consumes
    (odd PSUM group, even SBUF group) pairs at 1 result/cycle = 2 source
    elements/cycle. Per-pair accumulators land in a [128, 8] tile reduced
    once per block, keeping all fused ops independent for the scheduler.
Host work (O(N)): pose transform, fp16 feature split, final p^2 add +
sqrt + mean in float64, and the trivial non-symmetric branch.
"""

import numpy as np

N_PTS = 16384
N_CORES = 8
SYM_LIST = (0,)

PRED_PER_CORE = N_PTS // N_CORES          # 2048
N_BLOCKS = PRED_PER_CORE // 128           # 16 pred blocks of 128 rows
GROUP = 1024                              # gt points per PSUM group (2 banks)
N_GROUPS = N_PTS // GROUP                 # 16 groups -> 8 (even, odd) pairs
N_PAIRS = N_GROUPS // 2
N_CHAINS = N_PAIRS                        # one accumulator column per pair
K_ROWS = 11                               # fp16 split rows (3 per coord + 2)
LO_SCALE = np.float32(64.0)               # 2^6 subnormal-dodge scale

TRACE = False          # test.py sets True to capture a profiled run
LAST_RESULT = None     # BassKernelResults of the most recent device run

_COMPILED = None


def _register_min_ttr():
    """Register a custom fused DVE op:
        out = min(in0, in1);  accum_out = min(reduce_min(out), s0)
    One DVE instruction consumes TWO tiles at 1 result/cycle — 2x the
    throughput of tensor_reduce for the same reduction work. (The native
    TENSOR_TENSOR_REDUCE opcode crashes this runtime's exec unit; the
    table-driven custom-DVE path works.)"""
    from concourse.dve_spec import Spec, Src0, Src1, C0, minn, lower, _has_src1
    from concourse.dve_uop import DveOpSpec
    from concourse import dve_ops

    name = "MIN_TT_REDUCE_ANT"
    for o in dve_ops.OPS:
        if o.name == name:
            return o

    def _ref(in0, in1, c0, c1, c2):
        b = np.minimum(in0.astype(np.float32), in1.astype(np.float32))
        acc = np.minimum(
            np.float32(c0), b.reshape(b.shape[0], -1).min(axis=-1, keepdims=True)
        )
        return b, acc

    spec = Spec(body=minn(Src0, Src1), accum=minn, accum_init=C0, reference=_ref)
    row = max(dve_ops._SUB_OPCODE_FOR_NAME.values()) + 1
    dve_ops._SUB_OPCODE_FOR_NAME[name] = row
    shas = {}
    for ver in ("v3", "v4"):
        uops = lower(spec, ver=ver)
        shas[ver] = DveOpSpec(
            name=name, opcode=row, uops=uops, rd1_en=_has_src1(spec)
        ).sha(ver)
    op = dve_ops.DveOp(name, spec, subdim=False, uops_sha=shas)
    dve_ops.OPS.append(op)
    dve_ops.CUSTOM_DVE_SPECS[name] = spec
    return op


def _build_module():
    import concourse.bacc as bacc
    import concourse.tile as tile
    import concourse.mybir as mybir

    f16 = mybir.dt.float16
    f32 = mybir.dt.float32
    min_ttr = _register_min_ttr()

    nc = bacc.Bacc(
        "TRN2", target_bir_lowering=False, debug=False, num_devices=N_CORES
    )
    lhsT = nc.dram_tensor("lhsT", [K_ROWS, PRED_PER_CORE], f16, kind="ExternalInput")
    rhs = nc.dram_tensor("rhs", [K_ROWS, N_PTS], f16, kind="ExternalInput")
    out = nc.dram_tensor("out", [128, N_BLOCKS], f32, kind="ExternalOutput")

    with tile.TileContext(nc) as tc:
        with (
            tc.tile_pool(name="consts", bufs=1) as consts,
            tc.tile_pool(name="scrp", bufs=6) as scrp,
            tc.tile_pool(name="ttrop", bufs=4) as ttrop,
            tc.tile_pool(name="accp", bufs=12) as accp,
            tc.tile_pool(name="ps", bufs=4, space="PSUM") as psp,
        ):
            # features replicated at partition offsets 0/32/64/96 so four
            # K=11 matmuls run CONCURRENTLY in distinct PE row-groups.
            # rhs split into free-quarter tiles so early matmuls only
            # wait on the first quarter of the DMA.
            bounds = [0, 4096, 8192, 12288, N_PTS]
            lhs_sb = consts.tile([96 + K_ROWS, PRED_PER_CORE], f16)
            rhs_tiles = [
                consts.tile(
                    [96 + K_ROWS, bounds[q + 1] - bounds[q]],
                    f16,
                    name=f"rhs_sb{q}",
                )
                for q in range(len(bounds) - 1)
            ]
            outbuf = consts.tile([128, N_BLOCKS], f32)
            # every engine issues DMAs on its OWN hardware queue; all input
            # DMAs on one engine serialize (~20us). Spread the critical set
            # (lhs + first rhs quarter) across five engines' queues, then
            # round-robin the rest.
            engs = [nc.sync, nc.scalar, nc.gpsimd]
            nc.sync.dma_start(lhs_sb[0:K_ROWS, :], lhsT[:])
            nc.sync.dma_start(lhs_sb[32 : 32 + K_ROWS, :], lhsT[:])
            nc.scalar.dma_start(lhs_sb[64 : 64 + K_ROWS, :], lhsT[:])
            nc.scalar.dma_start(lhs_sb[96 : 96 + K_ROWS, :], lhsT[:])
            q0 = rhs_tiles[0]
            nc.gpsimd.dma_start(q0[0:K_ROWS, :], rhs[:, : bounds[1]])
            nc.sync.dma_start(q0[32 : 32 + K_ROWS, :], rhs[:, : bounds[1]])
            nc.scalar.dma_start(q0[64 : 64 + K_ROWS, :], rhs[:, : bounds[1]])
            nc.gpsimd.dma_start(q0[96 : 96 + K_ROWS, :], rhs[:, : bounds[1]])
            i = 0
            for q in range(1, len(bounds) - 1):
                for r in range(4):
                    p0 = 32 * r
                    engs[i % len(engs)].dma_start(
                        rhs_tiles[q][p0 : p0 + K_ROWS, :],
                        rhs[:, bounds[q] : bounds[q + 1]],
                    )
                    i += 1

            def rhs_slice(c):
                for q in range(len(bounds) - 1):
                    if c < bounds[q + 1]:
                        return rhs_tiles[q], c - bounds[q]
                raise AssertionError(c)

            # warm-up: absorb one-time ACT/DVE table-load penalties
            # while the DMAs stream (no dependency on inputs)
            warm = scrp.tile([128, 32], f32, tag="warm")
            warm2 = scrp.tile([128, 32], f32, tag="warm")
            wacc = accp.tile([128, 1], f32, tag="acc")
            nc.vector.memset(warm[:], 0.0)
            nc.scalar.copy(warm2[:], warm[:])
            nc.vector._custom_dve(
                min_ttr, out=warm2[:], in0=warm[:], in1=warm2[:],
                s0=3.0e38, accum_out=wacc[:],
            )

            n_mm = GROUP // 512

            def mm_group(ps, b, g):
                """One PSUM group: gt 512-tiles [n_mm*g, n_mm*(g+1)), spread
                over PE row-groups so matmuls run concurrently."""
                for t in range(n_mm):
                    j_tile = n_mm * g + t
                    p0 = 32 * (j_tile % 4)
                    src, co = rhs_slice(j_tile * 512)
                    nc.tensor.matmul(
                        ps[:, t * 512 : (t + 1) * 512],
                        lhs_sb[p0 : p0 + K_ROWS, b * 128 : (b + 1) * 128],
                        src[p0 : p0 + K_ROWS, co : co + 512],
                        start=True,
                        stop=True,
                        tile_position=(p0, 0),
                    )

            for b in range(N_BLOCKS):
                # every pair independent: partial mins land in a per-block
                # [128, N_CHAINS] tile, reduced once per block
                chain_accs = accp.tile([128, N_CHAINS], f32, tag="chacc")
                for k in range(N_PAIRS):
                    # even group: ScalarE copies PSUM -> SBUF
                    ps_a = psp.tile([128, GROUP], f32, tag="ps")
                    mm_group(ps_a, b, 2 * k)
                    scr = scrp.tile([128, GROUP], f32, tag="scr")
                    nc.scalar.copy(scr[:], ps_a[:])
                    # odd group: consumed straight from PSUM by the fused op
                    ps_b = psp.tile([128, GROUP], f32, tag="ps")
                    mm_group(ps_b, b, 2 * k + 1)
                    ttr_out = ttrop.tile([128, GROUP], f32, tag="ttro")
                    nc.vector._custom_dve(
                        min_ttr,
                        out=ttr_out[:],
                        in0=ps_b[:],
                        in1=scr[:],
                        s0=3.0e38,
                        accum_out=chain_accs[:, k : k + 1],
                    )
                nc.vector.tensor_reduce(
                    outbuf[:, b : b + 1],
                    chain_accs[:],
                    axis=mybir.AxisListType.X,
                    op=mybir.AluOpType.min,
                )
            nc.sync.dma_start(out[:], outbuf[:])
    nc.compile()
    return nc


def _get_module():
    global _COMPILED
    if _COMPILED is None:
        _COMPILED = _build_module()
    return _COMPILED


def _split_f16(x):
    """x (fp32) -> (hi, lo*2^6) fp16 pair with exact-product semantics."""
    hi = x.astype(np.float16)
    lo = ((x - hi.astype(np.float32)) * LO_SCALE).astype(np.float16)
    return hi, lo


def kernel(pred_R, pred_t, pts_model, pts_gt, model_index):
    global LAST_RESULT
    pred_R = np.asarray(pred_R, dtype=np.float32)
    pred_t = np.asarray(pred_t, dtype=np.float32)
    pts_model = np.asarray(pts_model, dtype=np.float32)
    pts_gt = np.asarray(pts_gt, dtype=np.float32)

    # pose transform (O(N), host): p[b,n,:] = R[b] @ model[b,n,:] + t[b]
    p = np.einsum("bij,bnj->bni", pred_R, pts_model) + pred_t[:, None, :]

    if int(model_index) not in SYM_LIST:
        diff = (p - pts_gt).astype(np.float64)
        loss = np.mean(np.sqrt(np.sum(diff * diff, axis=2)), axis=1)
        return loss.astype(np.float32)

    p = p[0]                       # (N, 3) queries
    g = pts_gt[0].astype(np.float32)   # (N, 3) references

    # features: S[i,j] = sum_k lhsT[k,i] * rhs[k,j] = -2 p.g + g^2
    a = -2.0 * p                                   # (N, 3)
    ah, al = _split_f16(a)
    gh, gl = _split_f16(g)
    c = (g.astype(np.float64) ** 2).sum(axis=1).astype(np.float32)   # g^2
    ch, cl = _split_f16(c)
    inv = np.float32(1.0) / LO_SCALE

    ones = np.ones(N_PTS, np.float16)
    # per coord: (Ah,Gh), (Al*64, Gh/64), (Ah/64, Gl*64); then (1,Ch), (1/64, Cl*64)
    lhs_rows, rhs_rows = [], []
    for ci in range(3):
        ahc = ah[:, ci]
        ghc = gh[:, ci]
        lhs_rows += [ahc, al[:, ci], (ahc.astype(np.float32) * inv).astype(np.float16)]
        rhs_rows += [ghc, (ghc.astype(np.float32) * inv).astype(np.float16), gl[:, ci]]
    lhs_rows += [ones, (ones.astype(np.float32) * inv).astype(np.float16)]
    rhs_rows += [ch, cl]
    lhs_full = np.stack(lhs_rows)                  # (11, N) fp16
    rhs_full = np.stack(rhs_rows)                  # (11, N) fp16

    nc = _get_module()
    from concourse.bass_utils import run_bass_kernel_spmd

    in_maps = []
    for core in range(N_CORES):
        sl = slice(core * PRED_PER_CORE, (core + 1) * PRED_PER_CORE)
        in_maps.append(
            {
                "lhsT": np.ascontiguousarray(lhs_full[:, sl]),
                "rhs": rhs_full,
            }
        )
    kw = {}
    if TRACE:
        kw = {"trace": True, "trace_cores": list(range(N_CORES))}
    res = run_bass_kernel_spmd(nc, in_maps, core_ids=list(range(N_CORES)), **kw)
    LAST_RESULT = res

    # assemble: out[p, b] = min_j S for pred index core*2048 + b*128 + p
    min_s = np.concatenate(
        [res.results[core]["out"].T.reshape(-1) for core in range(N_CORES)]
    ).astype(np.float64)
    p2 = (p.astype(np.float64) ** 2).sum(axis=1)
    d2 = np.maximum(p2 + min_s, 0.0)
    loss = np.mean(np.sqrt(d2))
    return np.array([loss], dtype=np.float32)


# revision 38
# speedup vs baseline: 1.0407x; 1.0109x over previous
"""Trainium2 kernel for nn_DistanceLoss (retrieval_knn, bs=1, N=16384).

reference semantics (sym branch, model_index in (0,)):
    p = R @ pts_model + t                      # (N, 3) predicted points
    d2[i, j] = ||p_i - g_j||^2                 # (N, N) vs ground-truth points
    loss = mean_i sqrt(min_j d2[i, j])         # scalar, shape (1,)

Key identity: sqrt(d2[i, argmin_j]) == sqrt(min_j d2[i, j]), so no
argmin/gather is needed — only a min-reduction over the distance matrix:
    min_j d2[i, j] = p_i^2 + min_j (g_j^2 - 2 p_i . g_j)

Device work (the O(N^2) part), sharded over 8 cores on the pred axis:
  - PE: S[i, j] = -2 p_i . g_j + g_j^2 as a K=11 matmul: each fp32 operand
    is split into fp16 hi/lo halves (lo scaled by 2^6 to dodge fp16
    subnormals, compensated on the other operand) so every partial product
    is exact in the fp32 PSUM accumulate; abs error ~1e-5.
  - Reduction of the 2048x16384 S-slice per core: ScalarE copies every even
    PSUM group to SBUF; a custom fused DVE op (MIN_TT_REDUCE_ANT:
    out = min(in0, in1), accum_out = min(s0, reduce_min(out))) consumes
    (odd PSUM group, even SBUF group) pairs at 1 result/cycle = 2 source
    elements/cycle. Per-pair accumulators land in a [128, 8] tile reduced
    once per block, keeping all fused ops independent for the scheduler.
Host work (O(N)): pose transform, fp16 feature split, final p^2 add +
sqrt + mean in float64, and the trivial non-symmetric branch.
"""

import numpy as np

N_PTS = 16384
N_CORES = 8
SYM_LIST = (0,)

PRED_PER_CORE = N_PTS // N_CORES          # 2048
N_BLOCKS = PRED_PER_CORE // 128           # 16 pred blocks of 128 rows
GROUP = 1024                              # gt points per PSUM group (2 banks)
N_GROUPS = N_PTS // GROUP                 # 16 groups -> 8 (even, odd) pairs
N_PAIRS = N_GROUPS // 2
N_CHAINS = N_PAIRS                        # one accumulator column per pair
K_ROWS = 11                               # fp16 split rows (3 per coord + 2)
LO_SCALE = np.float32(64.0)               # 2^6 subnormal-dodge scale

TRACE = False          # test.py sets True to capture a profiled run
LAST_RESULT = None     # BassKernelResults of the most recent device run

_COMPILED = None


def _register_min_ttr():
    """Register a custom fused DVE op:
        out = min(in0, in1);  accum_out = min(reduce_min(out), s0)
    One DVE instruction consumes TWO tiles at 1 result/cycle — 2x the
    throughput of tensor_reduce for the same reduction work. (The native
    TENSOR_TENSOR_REDUCE opcode crashes this runtime's exec unit; the
    table-driven custom-DVE path works.)"""
    from concourse.dve_spec import Spec, Src0, Src1, C0, minn, lower, _has_src1
    from concourse.dve_uop import DveOpSpec
    from concourse import dve_ops

    name = "MIN_TT_REDUCE_ANT"
    for o in dve_ops.OPS:
        if o.name == name:
            return o

    def _ref(in0, in1, c0, c1, c2):
        b = np.minimum(in0.astype(np.float32), in1.astype(np.float32))
        acc = np.minimum(
            np.float32(c0), b.reshape(b.shape[0], -1).min(axis=-1, keepdims=True)
        )
        return b, acc

    spec = Spec(body=minn(Src0, Src1), accum=minn, accum_init=C0, reference=_ref)
    row = max(dve_ops._SUB_OPCODE_FOR_NAME.values()) + 1
    dve_ops._SUB_OPCODE_FOR_NAME[name] = row
    shas = {}
    for ver in ("v3", "v4"):
        uops = lower(spec, ver=ver)
        shas[ver] = DveOpSpec(
            name=name, opcode=row, uops=uops, rd1_en=_has_src1(spec)
        ).sha(ver)
    op = dve_ops.DveOp(name, spec, subdim=False, uops_sha=shas)
    dve_ops.OPS.append(op)
    dve_ops.CUSTOM_DVE_SPECS[name] = spec
    return op


def _build_module():
    import concourse.bacc as bacc
    import concourse.tile as tile
    import concourse.mybir as mybir

    f16 = mybir.dt.float16
    f32 = mybir.dt.float32
    min_ttr = _register_min_ttr()

    nc = bacc.Bacc(
        "TRN2", target_bir_lowering=False, debug=False, num_devices=N_CORES
    )
    lhsT = nc.dram_tensor("lhsT", [K_ROWS, PRED_PER_CORE], f16, kind="ExternalInput")
    rhs = nc.dram_tensor("rhs", [K_ROWS, N_PTS], f16, kind="ExternalInput")
    out = nc.dram_tensor("out", [128, N_BLOCKS], f32, kind="ExternalOutput")

    with tile.TileContext(nc) as tc:
        with (
            tc.tile_pool(name="consts", bufs=1) as consts,
            tc.tile_pool(name="scrp", bufs=6) as scrp,
            tc.tile_pool(name="ttrop", bufs=4) as ttrop,
            tc.tile_pool(name="accp", bufs=12) as accp,
            tc.tile_pool(name="ps", bufs=4, space="PSUM") as psp,
        ):
            # features replicated at partition offsets 0/32/64/96 so four
            # K=11 matmuls run CONCURRENTLY in distinct PE row-groups.
            # rhs split into free-quarter tiles so early matmuls only
            # wait on the first quarter of the DMA.
            bounds = [0, 4096, 8192, 12288, N_PTS]
            lhs_sb = consts.tile([96 + K_ROWS, PRED_PER_CORE], f16)
            rhs_tiles = [
                consts.tile(
                    [96 + K_ROWS, bounds[q + 1] - bounds[q]],
                    f16,
                    name=f"rhs_sb{q}",
                )
                for q in range(len(bounds) - 1)
            ]
            outbuf = consts.tile([128, N_BLOCKS], f32)
            # every engine issues DMAs on its OWN hardware queue; all input
            # DMAs on one engine serialize (~20us). Spread the critical set
            # (lhs + first rhs quarter) across five engines' queues, then
            # round-robin the rest.
            # ScalarE must NOT issue input DMAs: each dma_start costs
            # ~900ns on the issuing sequencer, and ScalarE's first PSUM copy
            # is on the critical path. SP + GPSIMD sequencers are idle.
            engs = [nc.sync, nc.gpsimd]
            nc.sync.dma_start(lhs_sb[0:K_ROWS, :], lhsT[:])
            nc.gpsimd.dma_start(lhs_sb[32 : 32 + K_ROWS, :], lhsT[:])
            nc.sync.dma_start(lhs_sb[64 : 64 + K_ROWS, :], lhsT[:])
            nc.gpsimd.dma_start(lhs_sb[96 : 96 + K_ROWS, :], lhsT[:])
            q0 = rhs_tiles[0]
            nc.gpsimd.dma_start(q0[0:K_ROWS, :], rhs[:, : bounds[1]])
            nc.sync.dma_start(q0[32 : 32 + K_ROWS, :], rhs[:, : bounds[1]])
            nc.gpsimd.dma_start(q0[64 : 64 + K_ROWS, :], rhs[:, : bounds[1]])
            nc.sync.dma_start(q0[96 : 96 + K_ROWS, :], rhs[:, : bounds[1]])
            i = 0
            for q in range(1, len(bounds) - 1):
                for r in range(4):
                    p0 = 32 * r
                    engs[i % len(engs)].dma_start(
                        rhs_tiles[q][p0 : p0 + K_ROWS, :],
                        rhs[:, bounds[q] : bounds[q + 1]],
                    )
                    i += 1

            def rhs_slice(c):
                for q in range(len(bounds) - 1):
                    if c < bounds[q + 1]:
                        return rhs_tiles[q], c - bounds[q]
                raise AssertionError(c)

            # warm-up: absorb one-time ACT/DVE table-load penalties
            # while the DMAs stream (no dependency on inputs)
            warm = scrp.tile([128, 32], f32, tag="warm")
            warm2 = scrp.tile([128, 32], f32, tag="warm")
            wacc = accp.tile([128, 1], f32, tag="acc")
            nc.vector.memset(warm[:], 0.0)
            nc.scalar.copy(warm2[:], warm[:])
            nc.vector._custom_dve(
                min_ttr, out=warm2[:], in0=warm[:], in1=warm2[:],
                s0=3.0e38, accum_out=wacc[:],
            )

            n_mm = GROUP // 512

            def mm_group(ps, b, g):
                """One PSUM group: gt 512-tiles [n_mm*g, n_mm*(g+1)), spread
                over PE row-groups so matmuls run concurrently."""
                for t in range(n_mm):
                    j_tile = n_mm * g + t
                    p0 = 32 * (j_tile % 4)
                    src, co = rhs_slice(j_tile * 512)
                    nc.tensor.matmul(
                        ps[:, t * 512 : (t + 1) * 512],
                        lhs_sb[p0 : p0 + K_ROWS, b * 128 : (b + 1) * 128],
                        src[p0 : p0 + K_ROWS, co : co + 512],
                        start=True,
                        stop=True,
                        tile_position=(p0, 0),
                    )

            for b in range(N_BLOCKS):
                # every pair independent: partial mins land in a per-block
                # [128, N_CHAINS] tile, reduced once per block
                chain_accs = accp.tile([128, N_CHAINS], f32, tag="chacc")
                for k in range(N_PAIRS):
                    # even group: ScalarE copies PSUM -> SBUF
                    ps_a = psp.tile([128, GROUP], f32, tag="ps")
                    mm_group(ps_a, b, 2 * k)
                    scr = scrp.tile([128, GROUP], f32, tag="scr")
                    nc.scalar.copy(scr[:], ps_a[:])
                    # odd group: consumed straight from PSUM by the fused op
                    ps_b = psp.tile([128, GROUP], f32, tag="ps")
                    mm_group(ps_b, b, 2 * k + 1)
                    ttr_out = ttrop.tile([128, GROUP], f32, tag="ttro")
                    nc.vector._custom_dve(
                        min_ttr,
                        out=ttr_out[:],
                        in0=ps_b[:],
                        in1=scr[:],
                        s0=3.0e38,
                        accum_out=chain_accs[:, k : k + 1],
                    )
                nc.vector.tensor_reduce(
                    outbuf[:, b : b + 1],
                    chain_accs[:],
                    axis=mybir.AxisListType.X,
                    op=mybir.AluOpType.min,
                )
            nc.sync.dma_start(out[:], outbuf[:])
    nc.compile()
    return nc


def _get_module():
    global _COMPILED
    if _COMPILED is None:
        _COMPILED = _build_module()
    return _COMPILED


def _split_f16(x):
    """x (fp32) -> (hi, lo*2^6) fp16 pair with exact-product semantics."""
    hi = x.astype(np.float16)
    lo = ((x - hi.astype(np.float32)) * LO_SCALE).astype(np.float16)
    return hi, lo


def kernel(pred_R, pred_t, pts_model, pts_gt, model_index):
    global LAST_RESULT
    pred_R = np.asarray(pred_R, dtype=np.float32)
    pred_t = np.asarray(pred_t, dtype=np.float32)
    pts_model = np.asarray(pts_model, dtype=np.float32)
    pts_gt = np.asarray(pts_gt, dtype=np.float32)

    # pose transform (O(N), host): p[b,n,:] = R[b] @ model[b,n,:] + t[b]
    p = np.einsum("bij,bnj->bni", pred_R, pts_model) + pred_t[:, None, :]

    if int(model_index) not in SYM_LIST:
        diff = (p - pts_gt).astype(np.float64)
        loss = np.mean(np.sqrt(np.sum(diff * diff, axis=2)), axis=1)
        return loss.astype(np.float32)

    p = p[0]                       # (N, 3) queries
    g = pts_gt[0].astype(np.float32)   # (N, 3) references

    # features: S[i,j] = sum_k lhsT[k,i] * rhs[k,j] = -2 p.g + g^2
    a = -2.0 * p                                   # (N, 3)
    ah, al = _split_f16(a)
    gh, gl = _split_f16(g)
    c = (g.astype(np.float64) ** 2).sum(axis=1).astype(np.float32)   # g^2
    ch, cl = _split_f16(c)
    inv = np.float32(1.0) / LO_SCALE

    ones = np.ones(N_PTS, np.float16)
    # per coord: (Ah,Gh), (Al*64, Gh/64), (Ah/64, Gl*64); then (1,Ch), (1/64, Cl*64)
    lhs_rows, rhs_rows = [], []
    for ci in range(3):
        ahc = ah[:, ci]
        ghc = gh[:, ci]
        lhs_rows += [ahc, al[:, ci], (ahc.astype(np.float32) * inv).astype(np.float16)]
        rhs_rows += [ghc, (ghc.astype(np.float32) * inv).astype(np.float16), gl[:, ci]]
    lhs_rows += [ones, (ones.astype(np.float32) * inv).astype(np.float16)]
    rhs_rows += [ch, cl]
    lhs_full = np.stack(lhs_rows)                  # (11, N) fp16
    rhs_full = np.stack(rhs_rows)                  # (11, N) fp16

    nc = _get_module()
    from concourse.bass_utils import run_bass_kernel_spmd

    in_maps = []
    for core in range(N_CORES):
        sl = slice(core * PRED_PER_CORE, (core + 1) * PRED_PER_CORE)
        in_maps.append(
            {
                "lhsT": np.ascontiguousarray(lhs_full[:, sl]),
                "rhs": rhs_full,
            }
        )
    kw = {}
    if TRACE:
        kw = {"trace": True, "trace_cores": list(range(N_CORES))}
    res = run_bass_kernel_spmd(nc, in_maps, core_ids=list(range(N_CORES)), **kw)
    LAST_RESULT = res

    # assemble: out[p, b] = min_j S for pred index core*2048 + b*128 + p
    min_s = np.concatenate(
        [res.results[core]["out"].T.reshape(-1) for core in range(N_CORES)]
    ).astype(np.float64)
    p2 = (p.astype(np.float64) ** 2).sum(axis=1)
    d2 = np.maximum(p2 + min_s, 0.0)
    loss = np.mean(np.sqrt(d2))
    return np.array([loss], dtype=np.float32)
